# revision 24
# baseline (speedup 1.0000x reference)
"""Varlen causal attention (flash_attn_varlen semantics) on 8 Trainium2 cores.

Sharding: 16 heads across 8 cores (2 heads/core, Ulysses-style head shard,
identity comms). Each core runs the same SPMD Bass program on its head slice.

Per head the kernel computes S^T = (K^T)^T @ (Q^T) directly in the [k, q]
orientation, so P^T = exp(S^T * scale) lands in SBUF already transposed for
the PV matmul (lhsT = P^T block, rhs = V) -- no P transpose DMAs at all.
All matmuls run in bf16 (1 cycle/row on the PE). A ones-column prepended to
V yields the softmax denominator from the same PV accumulation. The
(causal x segment) block structure is specialized on the host from
cu_seqlens at trace time; segment-boundary and causal masking is done with
affine_select on the Pool engine over the exp'd P^T strips.
"""

import numpy as np

L = 4096
H = 16
D = 128
N_CORES = 8
H_PER_CORE = H // N_CORES
SCALE = 1.0 / float(np.sqrt(D))
QB = 128          # q/k block size
NB = L // QB      # 32 blocks
P_CAP = 188       # p ring capacity in 128-col blocks (47KB/partition bf16)


def _plan(cu: np.ndarray):
    """Host-side schedule from cu_seqlens, mirroring the reference
    searchsorted semantics exactly."""
    tok = np.arange(L)
    seg = np.searchsorted(cu[1:-1], tok, side="right")
    starts = np.concatenate([[0], cu[1:-1]])
    seg_start = starts[seg]

    # block-aligned causal/segment structure
    klo = [int(seg_start[i * QB]) // QB for i in range(NB)]          # per q block
    e = []                                                           # per k block
    for j in range(NB):
        mx = j
        for i in range(j, NB):
            if klo[i] <= j:
                mx = i
        e.append(mx)

    bnds = sorted(set(int(b) for b in cu[1:-1] if 0 < int(b) < L))
    bnd_in = {j: [b for b in bnds if j * QB < b < (j + 1) * QB] for j in range(NB)}
    # first boundary inside q block i (local offset), if any: queries >= that
    # boundary must not see ANY key from earlier k blocks
    bnd_first = {i: (bnd_in[i][0] - i * QB) for i in range(NB) if bnd_in[i]}

    # q tiles: greedy ranges [a, b) whose live P^T blocks fit the ring
    def live(a, b):
        s = 0
        for j in range(b):
            if e[j] >= a:
                s += min(e[j], b - 1) - max(j, a) + 1
        return s

    tiles = []
    a = 0
    while a < NB:
        b = a + 1
        while b < NB and live(a, b + 1) <= P_CAP:
            b += 1
        tiles.append((a, b))
        a = b
    return klo, e, bnd_in, bnd_first, tiles


def _build(cu: np.ndarray):
    import concourse.mybir as mybir
    import concourse.tile as tile
    from concourse import bacc
    from concourse.masks import make_identity

    f32 = mybir.dt.float32
    bf16 = mybir.dt.bfloat16
    AF = mybir.ActivationFunctionType
    GE = mybir.AluOpType.is_ge

    klo, e, bnd_in, bnd_first, tiles = _plan(cu)

    nc = bacc.Bacc("TRN2", target_bir_lowering=False, debug=False,
                   num_devices=N_CORES)
    q_d = nc.dram_tensor("q", [L, H_PER_CORE, D], f32, kind="ExternalInput")
    k_d = nc.dram_tensor("k", [L, H_PER_CORE, D], f32, kind="ExternalInput")
    v_d = nc.dram_tensor("v", [L, H_PER_CORE, D], f32, kind="ExternalInput")
    o_d = nc.dram_tensor("out", [L, H_PER_CORE, D], f32, kind="ExternalOutput")

    with tile.TileContext(nc) as tc:
        with (
            tc.tile_pool(name="consts", bufs=1) as consts,
            tc.tile_pool(name="pring", bufs=1) as pring,
            tc.tile_pool(name="big", bufs=2) as big,
            tc.tile_pool(name="io", bufs=3) as io,
            tc.tile_pool(name="ps", bufs=2, space="PSUM") as ps,
            tc.tile_pool(name="ops", bufs=2, space="PSUM") as ops,
        ):
            ident_bf = consts.tile([128, 128], bf16)
            make_identity(nc, ident_bf[:])
            warm = consts.tile([128, 1], f32)
            nc.scalar.activation(warm[:], ident_bf[:, 0:1], AF.Exp, scale=1.0)
            p_all = pring.tile([128, P_CAP, QB], bf16)

            # ---- prefetch input tiles (sync queue): q,k first, v later ----
            qin, kin, vin = {}, {}, {}
            for h in range(H_PER_CORE):
                for g in range(8):
                    t0 = g * 4 * QB
                    src = slice(t0, t0 + 4 * QB)
                    q_t = io.tile([128, 4, D], f32, tag="q_in")
                    nc.sync.dma_start(
                        q_t[:], q_d[src, h, :].rearrange("(u p) d -> p u d", p=128))
                    k_t = io.tile([128, 4, D], f32, tag="k_in")
                    nc.sync.dma_start(
                        k_t[:], k_d[src, h, :].rearrange("(u p) d -> p u d", p=128))
                    qin[h, g], kin[h, g] = q_t, k_t
                for g in range(8):
                    t0 = g * 4 * QB
                    src = slice(t0, t0 + 4 * QB)
                    v_t = io.tile([128, 4, D], f32, tag="v_in")
                    nc.sync.dma_start(
                        v_t[:], v_d[src, h, :].rearrange("(u p) d -> p u d", p=128))
                    vin[h, g] = v_t

            state = {h: {} for h in range(H_PER_CORE)}

            def prep_thunks(h):
                """Interleaved cast (DVE) + transpose (PE) + copy (DVE) units."""
                st = state[h]
                st["qbf"] = big.tile([128, NB, QB], bf16, tag="qbf", name="qbf")
                st["kbf"] = big.tile([128, NB, QB], bf16, tag="kbf", name="kbf")
                st["qt"] = big.tile([128, NB, QB], bf16, tag="qt", name="qt")
                st["kt"] = big.tile([128, NB, QB], bf16, tag="kt", name="kt")

                def cast(g):
                    def emit():
                        sl = slice(g * 4, (g + 1) * 4)
                        nc.vector.tensor_copy(st["qbf"][:, sl, :], qin[h, g][:])
                        nc.vector.tensor_copy(st["kbf"][:, sl, :], kin[h, g][:])
                    return emit

                def tr(which, G):
                    def emit():
                        src_t = st["qbf"] if which == "q" else st["kbf"]
                        dst = st["qt"] if which == "q" else st["kt"]
                        tp = ps.tile([128, 8, 128], bf16, tag="u", name="tp")
                        for u in range(8):
                            nc.tensor.transpose(tp[:, u, :],
                                                src_t[:, G * 8 + u, :],
                                                ident_bf[:])
                        nc.vector.tensor_copy(dst[:, G * 8:(G + 1) * 8, :], tp[:])
                    return emit

                casts = [cast(g) for g in range(8)]
                # (kind, G, fn): c0 c1 tq0 c2 tk0 c3 tq1 c4 tk1 ... tq3 tk3
                order = [("c", 0, casts[0]), ("c", 1, casts[1]),
                         ("tq", 0, tr("q", 0)), ("c", 2, casts[2]),
                         ("tk", 0, tr("k", 0)), ("c", 3, casts[3]),
                         ("tq", 1, tr("q", 1)), ("c", 4, casts[4]),
                         ("tk", 1, tr("k", 1)), ("c", 5, casts[5]),
                         ("tq", 2, tr("q", 2)), ("c", 6, casts[6]),
                         ("tk", 2, tr("k", 2)), ("c", 7, casts[7]),
                         ("tq", 3, tr("q", 3)), ("tk", 3, tr("k", 3))]
                return order

            def prep_v_thunks(h):
                st = state[h]
                vs = st["vs"] = big.tile([128, NB, 132], bf16, tag="vs", name="vs")

                def unit(g):
                    def emit():
                        if g == 0:
                            nc.vector.memset(vs[:, :, 0:1], 1.0)
                        nc.vector.tensor_copy(vs[:, g * 4:(g + 1) * 4, 1:129],
                                              vin[h, g][:])
                    return emit
                return [unit(g) for g in range(8)]

            def emit_masks(j, qsb, qeb, off):
                n = qeb - qsb + 1
                if qsb == j:
                    nc.gpsimd.affine_select(
                        out=p_all[:, off, :], in_=p_all[:, off, :],
                        compare_op=GE, fill=0.0, base=0,
                        pattern=[[1, QB]], channel_multiplier=-1)
                for bnd in bnd_in[j]:
                    r = bnd - j * QB
                    if qsb == j:
                        nc.gpsimd.affine_select(
                            out=p_all[:, off, r:QB], in_=p_all[:, off, r:QB],
                            compare_op=GE, fill=0.0, base=-r,
                            pattern=[[0, QB - r]], channel_multiplier=1)
                        if n > 1:
                            nc.gpsimd.affine_select(
                                out=p_all[:, off + 1:off + n, :],
                                in_=p_all[:, off + 1:off + n, :],
                                compare_op=GE, fill=0.0, base=-r,
                                pattern=[[0, n - 1], [0, QB]],
                                channel_multiplier=1)
                    else:
                        nc.gpsimd.affine_select(
                            out=p_all[:, off:off + n, :],
                            in_=p_all[:, off:off + n, :],
                            compare_op=GE, fill=0.0, base=-r,
                            pattern=[[0, n], [0, QB]],
                            channel_multiplier=1)
                for i in range(max(qsb, j + 1), qeb + 1):
                    r = bnd_first.get(i)
                    if r is not None:
                        nc.gpsimd.memset(p_all[:, off + (i - qsb), r:QB], 0.0)

            CHUNK = 12

            def qk_thunks(h, a, b, parts):
                """One thunk per 16-block S^T chunk: matmuls + exp + masks."""
                seq = []
                for j, (qsb, qeb, off) in parts.items():
                    for i in range(qsb, qeb + 1):
                        seq.append((j, i, i == qeb))

                def chunk(pos, nblk):
                    box = {}

                    def mm():
                        qt, kt = state[h]["qt"], state[h]["kt"]
                        sp = ps.tile([128, CHUNK, 128], f32, tag="u", name="sp")
                        box["sp"] = sp
                        r0 = 0
                        while r0 < nblk:
                            j0 = seq[pos + r0][0]
                            lim = (r0 // 4 + 1) * 4
                            r1 = r0 + 1
                            while (r1 < nblk and r1 < lim and
                                   seq[pos + r1][0] == j0):
                                r1 += 1
                            qs0 = seq[pos + r0][1]
                            nc.tensor.matmul(
                                sp[:, r0:r1, :], kt[:, j0, :],
                                qt[:, qs0:qs0 + (r1 - r0), :],
                                start=True, stop=True)
                            r0 = r1

                    def post():
                        nc.scalar.activation(
                            p_all[:, pos:pos + nblk, :],
                            box["sp"][:, 0:nblk, :], AF.Exp, scale=SCALE)
                        for t in range(nblk):
                            j, i, is_end = seq[pos + t]
                            if is_end:
                                emit_masks(j, *parts[j])
                    return mm, post

                out = []
                meta = []
                pos = 0
                done = -1
                ramp = [4, 8]  # small leading chunks: earlier first exp
                while pos < len(seq):
                    nblk = min(ramp.pop(0) if ramp else CHUNK,
                               len(seq) - pos)
                    out.append(chunk(pos, nblk))  # (mm, post) pair
                    for t in range(nblk):
                        j, i, is_end = seq[pos + t]
                        if is_end:
                            done = max(done, j)
                    meta.append((max(s[1] for s in seq[pos:pos + nblk]),
                                 max(s[0] for s in seq[pos:pos + nblk]),
                                 done, (pos, pos + nblk)))
                    pos += nblk
                return out, meta

            def pv_group_thunks(h, a, b, parts):
                """One thunk per <=3-block PV group: accumulate, norm, store.
                Returns [(req_strip, req_vblock, fn), ...]."""
                groups = []
                i = a
                while i < b:
                    i1 = min(i + 3, b)

                    def grp(i0, i1):
                        def emit():
                            vs = state[h]["vs"]
                            op = ops.tile([128, 3, 129], f32, tag="o",
                                          name="op")
                            for i in range(i0, i1):
                                u3 = i - i0
                                jlo = klo[i]
                                for j in range(jlo, i + 1):
                                    qsb, qeb, off = parts[j]
                                    idx = off + (i - qsb)
                                    nc.tensor.matmul(
                                        op[:, u3, :], p_all[:, idx, :],
                                        vs[:, j, 0:129],
                                        start=(j == jlo), stop=(j == i))
                            nn = i1 - i0
                            rc = io.tile([128, 3, 1], f32, tag="rc")
                            nc.vector.reciprocal(rc[:, 0:nn, :],
                                                 op[:, 0:nn, 0:1])
                            o3 = io.tile([128, 3, D], f32, tag="o3")
                            nc.vector.tensor_mul(
                                o3[:, 0:nn, :], op[:, 0:nn, 1:129],
                                rc[:, 0:nn, :].broadcast_to([128, nn, D]))
                            nc.sync.dma_start(
                                o_d[i0 * QB:i1 * QB, h, :]
                                .rearrange("(u p) d -> p u d", p=128),
                                o3[:, 0:nn, :])
                        return emit

                    groups.append((i1 - 1, (i1 - 1) // 4, grp(i, i1)))
                    i = i1
                return groups

            def mk_parts(a, b):
                parts = {}
                off = 0
                for j in range(b):
                    if e[j] < a:
                        continue
                    qsb = max(j, a)
                    qeb = min(e[j], b - 1)
                    parts[j] = (qsb, qeb, off)
                    off += qeb - qsb + 1
                return parts

            def run_units(units, bg=()):
                pending = list(bg)
                for u in units:
                    u()
                    if pending:
                        pending.pop(0)()
                for fn in pending:
                    fn()

            # schedule: prep(h+1) rides inside qk(h); qk(h+1) rides inside
            # pv(h) -- the Act engine (exp) never waits for a phase change
            seq_ht = [(h, ti) for h in range(H_PER_CORE)
                      for ti in range(len(tiles))]
            parts_of = {}
            for h, ti in seq_ht:
                a, b = tiles[ti]
                parts_of[h, ti] = mk_parts(a, b)

            HOIST = int(__import__("os").environ.get("KHOIST", "4"))

            def ring_reader(parts):
                rd = {}
                for j, (qsb, qeb, off) in parts.items():
                    for i in range(qsb, qeb + 1):
                        rd[off + (i - qsb)] = i
                return rd

            plans = {}

            def get_plan(h, ti):
                if (h, ti) not in plans:
                    a, b = tiles[ti]
                    parts = parts_of[h, ti]
                    chunks, metas = qk_thunks(h, a, b, parts)
                    groups = pv_group_thunks(h, a, b, parts)
                    plans[h, ti] = (chunks, metas, groups)
                return plans[h, ti]

            def emit_unit(h, ti, prep_units, bg, hoisted):
                """Emit one (head, tile) unit: remaining qk chunks, prep units
                (gated by chunk needs), vs casts and pv groups (gated by
                strip completion), plus background thunks with ring reqs."""
                a, b = tiles[ti]
                parts = parts_of[h, ti]
                chunks, metas, groups = get_plan(h, ti)
                vsu = state[h].pop("_vs_units", None)
                if vsu is None:
                    vsu = prep_v_thunks(h)
                pu = list(prep_units or [])
                pos_of = {(k, G): i for i, (k, G, _) in enumerate(pu)}
                pending = list(bg)
                pi = 0
                gi = 0
                vi = 0

                def flush_prep(upto):
                    nonlocal pi
                    while pi <= upto:
                        pu[pi][2]()
                        pi += 1

                def pop_bg(i_last, free_pops):
                    while pending and pending[0][0] <= i_last:
                        if pending[0][0] < 0:
                            if free_pops == 0:
                                break
                            free_pops -= 1
                        req, prep_req, fn = pending.pop(0)
                        if pu and prep_req >= 0:
                            flush_prep(prep_req)
                        fn()

                def emit_groups(done):
                    nonlocal gi, vi
                    while gi < len(groups) and groups[gi][0] <= done:
                        while vi <= min(groups[gi][1], len(vsu) - 1):
                            vsu[vi]()
                            vi += 1
                        i_last = groups[gi][0]
                        groups[gi][2]()
                        gi += 1
                        pop_bg(i_last, 3)  # pace ungated (req -1) thunks

                prev_post = None
                prev_done = -1
                if hoisted:
                    prev_done = metas[hoisted - 1][2]
                for ci in range(hoisted, len(chunks)):
                    mm, post = chunks[ci]
                    maxi, maxj, done, _ = metas[ci]
                    if pu:
                        flush_prep(max(pos_of[("tq", maxi // 8)],
                                       pos_of[("tk", maxj // 8)]))
                    mm()
                    if prev_post is not None:
                        prev_post()
                    emit_groups(prev_done)
                    prev_post, prev_done = post, done
                if prev_post is not None:
                    prev_post()
                if pu:
                    flush_prep(len(pu) - 1)
                emit_groups(NB)
                while vi < len(vsu):
                    vsu[vi]()
                    vi += 1
                state[h]["_vs_units"] = []
                for _, prep_req, fn in pending:
                    fn()

            prep_done = {}
            hoisted_of = {}
            for idx, (h, ti) in enumerate(seq_ht):
                parts = parts_of[h, ti]
                pu = None
                if not prep_done.get(h):
                    pu = prep_thunks(h)
                    prep_done[h] = True
                bg = []
                nxt = seq_ht[idx + 1] if idx + 1 < len(seq_ht) else None
                if nxt is not None:
                    nh, nti = nxt
                    if nti == 0 and not prep_done.get(nh):
                        bg += [(-1, -1, u[2]) for u in prep_thunks(nh)]
                        state[nh]["_vs_units"] = prep_v_thunks(nh)
                        prep_done[nh] = True
                    # hoist the next unit's first chunks, gated on this
                    # unit's last read of the ring region they overwrite
                    nchunks, nmetas, _ = get_plan(nh, nti)
                    rd = ring_reader(parts)
                    nh_count = min(HOIST, len(nchunks))
                    req_run = -1
                    for ci in range(nh_count):
                        mm, post = nchunks[ci]
                        maxi, maxj, _, (p0, p1) = nmetas[ci]
                        req = max((rd.get(r, -1) for r in range(p0, p1)),
                                  default=-1)
                        req_run = max(req_run, req)
                        prep_req = -1
                        if nh == h:
                            # prep order: tq(G) at 2+4G, tk(G) at min(4+4G,15)
                            prep_req = min(15, max(2 + 4 * (maxi // 8),
                                                   4 + 4 * (maxj // 8)))

                        def both(mm=mm, post=post):
                            mm()
                            post()
                        bg.append((req_run, prep_req, both))
                    hoisted_of[nh, nti] = nh_count
                emit_unit(h, ti, pu, bg, hoisted_of.get((h, ti), 0))

    nc.compile()
    return nc


def _interleave(a, b):
    out = []
    ia = ib = 0
    while ia < len(a) or ib < len(b):
        if ia < len(a):
            out.append(a[ia]); ia += 1
        if ib < len(b):
            out.append(b[ib]); ib += 1
    return out


def _run(query, key, value, cu_seqlens, trace=False, **spmd_kwargs):
    from concourse import bass_utils

    query = np.ascontiguousarray(np.asarray(query, dtype=np.float32))
    key = np.ascontiguousarray(np.asarray(key, dtype=np.float32))
    value = np.ascontiguousarray(np.asarray(value, dtype=np.float32))
    cu = np.asarray(cu_seqlens, dtype=np.int64)

    nc = _build(cu)
    in_maps = []
    for c in range(N_CORES):
        hs = slice(c * H_PER_CORE, (c + 1) * H_PER_CORE)
        in_maps.append({
            "q": np.ascontiguousarray(query[:, hs, :]),
            "k": np.ascontiguousarray(key[:, hs, :]),
            "v": np.ascontiguousarray(value[:, hs, :]),
        })
    res = bass_utils.run_bass_kernel_spmd(nc, in_maps, list(range(N_CORES)),
                                          trace=trace, **spmd_kwargs)
    out = np.empty((L, H, D), dtype=np.float32)
    for c in range(N_CORES):
        out[:, c * H_PER_CORE:(c + 1) * H_PER_CORE, :] = res.results[c]["out"]
    return out, res


def kernel(query, key, value, cu_seqlens):
    out, _ = _run(query, key, value, cu_seqlens)
    return out
